# revision 20
# baseline (speedup 1.0000x reference)
"""DenseGINConv on 8 TRN2 NeuronCores (v4: superblock gathers, big SWDGE calls).

  agg = segment_sum(x[edge_src], edge_dst, N)        # gather + scatter-add
  h   = (1+eps)*x + agg
  out = relu(relu(relu(h @ W1 + b1) @ W2 + b2) + bias)

Strategy (fully SPMD, zero collectives):
  - Shard edges by dst range: core i owns dst nodes [i*12500, (i+1)*12500).
  - Replicate x as an fp16 gather table in every core's HBM; gather src rows
    with the dma_gather GPSIMD ucode (int16 indices -> 4 table chunks).
  - Dst nodes are balanced into NBLK=100 blocks of 128 slots, grouped into
    5 superblocks of 20 blocks. Edges are ordered (super, chunk, block) and
    padded to 128-edge groups per (block, chunk).
  - v3 issued one small dma_gather per (block, chunk) [416 calls/core]; the
    ~1us fixed SWDGE cost per call made GPSIMD the bottleneck (~486us busy
    in TimelineSim). v4 enlarges the descriptor ring
    (dynamic_dma_scratch_size=32768 -> 2048-desc rings, 4MB SBUF) and issues
    14-column (1792-edge) calls spanning blocks within a (super, chunk)
    piece: ~120 calls/core, ~190us GPSIMD, leaving the DMA engines (~330us)
    as the bottleneck.
  - PSUM is bank-granular: each of 5 banks holds a [128, 512] f32 tile = 4
    blocks' agg^T accumulated via matmul sub-range outputs (subtile deps).
    One-hot columns are built per piece in one tensor_tensor (is_equal vs
    iota); pieces alternate between DVE and GPSIMD to balance engine load.
  - After a super's last piece: hT = agg + (1+eps)x added in place into the
    fp16 xsT buffer (DVE), then the fp16 2-layer MLP + final relu per block,
    overlapped with the next super's gathers. outT is written fp16 and
    upcast on the host.
"""

import math

import numpy as np

import concourse.bacc as bacc
import concourse.mybir as mybir
import concourse.tile as tile
from concourse.bass_utils import run_bass_kernel_spmd
from concourse.library_config import mlp as mlp_lib

N = 100000
C = 128
M = 8            # cores
NPC = N // M     # nodes per core = 12500
BLK = 128                       # dst slots per block
NBLK = 104                      # dst blocks / core (6.5% cap slack)
SUPER = 13                      # blocks per superblock (4 psum bank tiles)
NSUP = NBLK // SUPER            # 8 superblocks
QUAD = 4                        # blocks per psum bank tile
SLOTS = NBLK * BLK              # padded dst slots / core = 13312
P = 128
NCH = 4                         # x-table chunks (int16 index range)
CH = math.ceil((N + 1) / NCH)   # rows per chunk (25001 <= 32768)
MAXCOLS_CALL = 7                # cols per dma_gather (64-entry rings)
SCRATCH = 16384                 # dynamic_dma_scratch_size (default)
LIGHT = 8                       # trailing blocks with 3-col (384-edge) caps

f32 = mybir.dt.float32
f16 = mybir.dt.float16
i16 = mybir.dt.int16

_cache = {}


def build(Mmat):
    """Build the per-core Bass program. Mmat[b][c] = 128-edge columns for
    (dst-block b, x-chunk c); identical across cores."""
    nc = bacc.Bacc(
        "TRN2", target_bir_lowering=False, debug=False, enable_asserts=True,
        num_swdge_queues=4, dynamic_dma_scratch_size=SCRATCH,
    )
    totcol = int(Mmat.sum())
    sum16 = totcol * 8  # idx columns (int16, 16-wrapped): 128/16 per column

    xt = nc.dram_tensor("xt", [NCH * CH, C], f16, kind="ExternalInput")
    srcs = nc.dram_tensor("srcs", [P, sum16], i16, kind="ExternalInput")
    dstl = nc.dram_tensor("dstl", [P, totcol], f16, kind="ExternalInput")
    xsT = nc.dram_tensor("xsT", [P, SLOTS], f16, kind="ExternalInput")
    w1 = nc.dram_tensor("W1", [C, C], f16, kind="ExternalInput")
    w2 = nc.dram_tensor("W2", [C, C], f16, kind="ExternalInput")
    b1 = nc.dram_tensor("b1c", [C, 1], f32, kind="ExternalInput")
    b2 = nc.dram_tensor("b2c", [C, 1], f32, kind="ExternalInput")
    bias = nc.dram_tensor("biasc", [C, 1], f32, kind="ExternalInput")
    iota = nc.dram_tensor("iota", [P, BLK], f16, kind="ExternalInput")
    outT = nc.dram_tensor("outT", [P, SLOTS], f16, kind="ExternalOutput")

    piece_cols = {
        (s, c): int(sum(Mmat[b][c] for b in range(s * SUPER, (s + 1) * SUPER)))
        for s in range(NSUP)
        for c in range(NCH)
    }
    maxpc = max(piece_cols.values())

    with tile.TileContext(nc) as tc:
        with (
            tc.tile_pool(name="const", bufs=1) as cp,
            tc.tile_pool(name="gath", bufs=3) as gp,
            tc.tile_pool(name="oh", bufs=3) as op,
            tc.tile_pool(name="mlp", bufs=3) as mp,
            tc.tile_pool(name="psA", bufs=1, space="PSUM") as psA,
            tc.tile_pool(name="psB", bufs=1, space="PSUM") as psB,
            tc.tile_pool(name="psC", bufs=1, space="PSUM") as psC,
        ):
            nc.gpsimd.load_library(mlp_lib)
            srcs_sb = cp.tile([P, sum16], i16)
            nc.sync.dma_start(srcs_sb[:], srcs[:])
            dstl_sb = cp.tile([P, totcol], f16)
            nc.sync.dma_start(dstl_sb[:], dstl[:])
            xsT_sb = cp.tile([P, SLOTS], f16)
            nc.sync.dma_start(xsT_sb[:], xsT[:])
            w1_sb = cp.tile([C, C], f16)
            nc.sync.dma_start(w1_sb[:], w1[:])
            w2_sb = cp.tile([C, C], f16)
            nc.sync.dma_start(w2_sb[:], w2[:])
            b1_sb = cp.tile([C, 1], f32)
            nc.sync.dma_start(b1_sb[:], b1[:])
            b2_sb = cp.tile([C, 1], f32)
            nc.sync.dma_start(b2_sb[:], b2[:])
            bias_sb = cp.tile([C, 1], f32)
            nc.sync.dma_start(bias_sb[:], bias[:])
            iota_sb = cp.tile([P, BLK], f16)
            nc.sync.dma_start(iota_sb[:], iota[:])

            col = 0      # global column counter (piece-ordered layout)
            seg16 = 0    # idx column counter
            qn = 0
            for s in range(NSUP):
                blocks = range(s * SUPER, (s + 1) * SUPER)
                # PSUM zero regions are bank-wide: start/stop once per quad
                # bank per super (first/last matmul into it); lanes auto-zero
                # on first touch via the pending-zero mechanism.
                nquad = (SUPER + QUAD - 1) // QUAD
                qtot = {
                    q: int(
                        sum(
                            Mmat[b][c]
                            for c in range(NCH)
                            for b in blocks
                            if (b - s * SUPER) // QUAD == q
                        )
                    )
                    for q in range(nquad)
                }
                qseen = {q: 0 for q in range(nquad)}
                # 5 bank tiles of 4 blocks each; block b -> quad q, lane l
                quads = {}
                for q in range(nquad):
                    quads[q] = psA.tile(
                        [P, QUAD * BLK], f32, tag=f"aggq{q}", name=f"aggq{q}"
                    )
                for c in range(NCH):
                    pc = piece_cols[(s, c)]
                    if pc == 0:
                        continue
                    gb = gp.tile([P, maxpc * C], f16, tag="g")
                    coff = 0
                    while coff < pc:
                        mk = min(pc - coff, MAXCOLS_CALL)
                        ni = mk * 128
                        nc.gpsimd.dma_gather(
                            gb[:, coff * C:(coff + mk) * C].rearrange(
                                "p (k e) -> p k e", e=C
                            ),
                            xt[c * CH:(c + 1) * CH, :],
                            srcs_sb[:, seg16:seg16 + ni // 16],
                            ni, ni, C, queue_num=qn % 4,
                        )
                        qn += 1
                        seg16 += ni // 16
                        coff += mk
                    oh = op.tile([P, maxpc * BLK], f16, tag="oh")
                    # (is_equal TensorTensor is DVE-only: the Pool engine
                    # fails the neuronxcc ISA opcode check for it.)
                    nc.vector.tensor_tensor(
                        out=oh[:, :pc * BLK].rearrange("p (m e) -> p m e", e=BLK),
                        in0=dstl_sb[:, col:col + pc]
                        .rearrange("p (m o) -> p m o", o=1)
                        .to_broadcast([P, pc, BLK]),
                        in1=iota_sb[:]
                        .rearrange("p (o e) -> p o e", o=1)
                        .to_broadcast([P, pc, BLK]),
                        op=mybir.AluOpType.is_equal,
                    )
                    goff = 0
                    for b in blocks:
                        mb = int(Mmat[b][c])
                        bl = b - s * SUPER
                        q, lane = bl // QUAD, bl % QUAD
                        for j in range(mb):
                            nc.tensor.matmul(
                                out=quads[q][:, lane * BLK:(lane + 1) * BLK],
                                lhsT=gb[:, (goff + j) * C:(goff + j + 1) * C],
                                rhs=oh[:, (goff + j) * BLK:(goff + j + 1) * BLK],
                                start=(qseen[q] == 0),
                                stop=(qseen[q] == qtot[q] - 1),
                            )
                            qseen[q] += 1
                        goff += mb
                    col += pc
                # hT = agg + (1+eps) x, in place over xsT; then MLP per block
                ps1 = ps2 = None
                for b in blocks:
                    bl = b - s * SUPER
                    q, lane = bl // QUAD, bl % QUAD
                    bcols = slice(b * BLK, (b + 1) * BLK)
                    nc.vector.tensor_add(
                        out=xsT_sb[:, bcols],
                        in0=quads[q][:, lane * BLK:(lane + 1) * BLK],
                        in1=xsT_sb[:, bcols],
                    )
                    if lane == 0:
                        ps1 = psB.tile([P, QUAD * BLK], f32, tag="ps1")
                        ps2 = psC.tile([P, QUAD * BLK], f32, tag="ps2")
                    p1 = ps1[:, lane * BLK:(lane + 1) * BLK]
                    nc.tensor.matmul(
                        out=p1, lhsT=w1_sb[:], rhs=xsT_sb[:, bcols],
                        start=True, stop=True,
                    )
                    h1 = mp.tile([P, BLK], f16, tag="h1")
                    nc.scalar.activation(
                        h1[:], p1, mybir.ActivationFunctionType.Relu,
                        bias=b1_sb[:],
                    )
                    p2 = ps2[:, lane * BLK:(lane + 1) * BLK]
                    nc.tensor.matmul(
                        out=p2, lhsT=w2_sb[:], rhs=h1[:], start=True, stop=True
                    )
                    h2 = mp.tile([P, BLK], f16, tag="h2")
                    nc.scalar.activation(
                        h2[:], p2, mybir.ActivationFunctionType.Relu,
                        bias=b2_sb[:],
                    )
                    ob = mp.tile([P, BLK], f16, tag="ob")
                    nc.scalar.activation(
                        ob[:], h2[:], mybir.ActivationFunctionType.Relu,
                        bias=bias_sb[:],
                    )
                    nc.sync.dma_start(out=outT[:, bcols], in_=ob[:])

    nc.compile()
    return nc


def prep(x, edge_src, edge_dst, eps):
    """Host-side sharding -> per-core (srcs16, dstl, xsT) + shared table/M."""
    x = np.asarray(x, dtype=np.float32)
    edge_src = np.asarray(edge_src).astype(np.int64)
    edge_dst = np.asarray(edge_dst).astype(np.int64)
    epsv = float(np.asarray(eps).reshape(-1)[0])

    core = edge_dst // NPC
    dst_local = edge_dst - core * NPC
    chunk = edge_src // CH
    lidx = (edge_src - chunk * CH).astype(np.int16)

    percore = []
    pos_list = []
    counts = np.zeros((M, NBLK, NCH), dtype=np.int64)
    for i in range(M):
        sel = core == i
        dl, c_i = dst_local[sel], chunk[sel]
        deg = np.bincount(dl * NCH + c_i, minlength=NPC * NCH).reshape(NPC, NCH)
        # Mostly 4-column (512-edge) caps per (block, chunk), with LIGHT
        # trailing blocks capped at 3 columns: Mmat is the max over cores, so
        # shared sub-512 quotas shave ~4% of the padded gather volume.
        caps = np.full((NBLK, NCH), 4 * 128, dtype=np.int64)
        caps[NBLK - LIGHT:] = 3 * 128
        dblk, dslot = _balance(deg, NBLK, BLK, caps)
        pos_list.append(dblk * BLK + dslot)
        b_i = dblk[dl]
        slot_i = dslot[dl]
        sup_i = b_i // SUPER
        order = np.lexsort((b_i, c_i, sup_i))   # (super, chunk, block)
        percore.append((lidx[sel][order], slot_i[order],
                        b_i[order], c_i[order]))
        cnt = np.bincount(b_i * NCH + c_i, minlength=NBLK * NCH)
        counts[i] = cnt.reshape(NBLK, NCH)

    Mmat = np.ceil(counts.max(axis=0) / 128).astype(np.int64)  # [NBLK, NCH]
    totcol = int(Mmat.sum())

    # column-start offset of each (b, c) in the (super, chunk, block) layout
    ksort = np.array(
        [b * NCH + c
         for s in range(NSUP)
         for c in range(NCH)
         for b in range(s * SUPER, (s + 1) * SUPER)],
        dtype=np.int64,
    )
    colstart = np.zeros(NBLK * NCH, dtype=np.int64)
    acc = 0
    for kk in ksort:
        colstart[kk] = acc
        acc += Mmat.reshape(-1)[kk]
    assert acc == totcol

    srcs_list, dstl_list, xsT_list = [], [], []
    for i in range(M):
        li, sl, b_i, c_i = percore[i]
        key = b_i * NCH + c_i
        kcnt = counts[i].reshape(-1)
        # edges are sorted (super, chunk, block): cumulate in that order
        pos_in_sorted = np.zeros(NBLK * NCH, dtype=np.int64)
        run = 0
        for kk in ksort:
            pos_in_sorted[kk] = run
            run += kcnt[kk]
        pos = np.arange(len(li)) - pos_in_sorted[key]
        gpos = colstart[key] * 128 + pos  # position in the padded edge stream

        v = np.zeros(totcol * 128, dtype=np.int16)   # pad: row 0 of chunk
        d = np.full(totcol * 128, -1.0, dtype=np.float16)
        v[gpos] = li
        d[gpos] = sl

        # idx stream wraps per 16 within each call; calls are whole columns
        # and 128 % 16 == 0, so the wrap is stream-global as before.
        w = v.reshape(-1, 16).T.copy()               # [16, totcol*8]
        srcs_list.append(np.tile(w, (8, 1)))
        dstl_list.append(
            np.ascontiguousarray(d.reshape(totcol, 128).T)  # [128, totcol]
        )
        xs = np.zeros((P, SLOTS), dtype=np.float16)
        xs[:, pos_list[i]] = (
            ((1.0 + epsv) * x[i * NPC:(i + 1) * NPC]).astype(np.float16).T
        )
        xsT_list.append(xs)

    xt = np.zeros((NCH * CH, C), dtype=np.float16)
    xt[:N] = x
    return Mmat, srcs_list, dstl_list, xsT_list, xt, pos_list


def _balance(deg, nbins, cap_slots, cap_edges):
    """Best-fit-decreasing: assign dsts (rows of deg [ND, NCH]) to nbins
    blocks, <= cap_slots dsts and (soft) <= cap_edges[b, c] edges each."""
    nd = deg.shape[0]
    tot = deg.sum(axis=1)
    order = np.argsort(-tot, kind="stable")
    sums = np.zeros((nbins, deg.shape[1]), dtype=np.int64)
    load = np.zeros(nbins, dtype=np.int64)
    cnt = np.zeros(nbins, dtype=np.int64)
    blk = np.empty(nd, dtype=np.int64)
    slot = np.empty(nd, dtype=np.int64)
    big = 1 << 50
    for d in order:
        v = deg[d]
        ok = (cnt < cap_slots) & ((sums + v) <= cap_edges).all(axis=1)
        if ok.any():
            b = int(np.argmin(np.where(ok, load, big)))  # LPT: least-loaded fit
        else:
            over = np.maximum(sums + v - cap_edges, 0).sum(axis=1)
            over[cnt >= cap_slots] = big
            b = int(np.argmin(over))
        blk[d] = b
        slot[d] = cnt[b]
        cnt[b] += 1
        load[b] += tot[d]
        sums[b] += v
    return blk, slot


def make_in_maps(inputs):
    Mmat, srcs_list, dstl_list, xsT_list, xt, pos_list = prep(
        inputs["x"], inputs["edge_src"], inputs["edge_dst"], inputs["eps"]
    )
    w1 = np.ascontiguousarray(np.asarray(inputs["W1"], dtype=np.float16))
    w2 = np.ascontiguousarray(np.asarray(inputs["W2"], dtype=np.float16))
    b1c = np.asarray(inputs["b1"], dtype=np.float32).reshape(C, 1)
    b2c = np.asarray(inputs["b2"], dtype=np.float32).reshape(C, 1)
    biasc = np.asarray(inputs["bias"], dtype=np.float32).reshape(C, 1)
    iota = np.tile(np.arange(BLK, dtype=np.float16), (P, 1))
    in_maps = [
        dict(
            xt=xt, srcs=srcs_list[i], dstl=dstl_list[i], xsT=xsT_list[i],
            W1=w1, W2=w2, b1c=b1c, b2c=b2c, biasc=biasc, iota=iota,
        )
        for i in range(M)
    ]
    return Mmat, in_maps, pos_list


def get_program(Mmat):
    key = Mmat.tobytes()
    if key not in _cache:
        _cache[key] = build(Mmat)
    return _cache[key]


def assemble(results, pos_list):
    out = np.empty((N, C), dtype=np.float32)
    for i in range(M):
        out[i * NPC:(i + 1) * NPC] = (
            results[i]["outT"].astype(np.float32).T[pos_list[i]]
        )
    return out


def kernel(**inputs) -> np.ndarray:
    Mmat, in_maps, pos_list = make_in_maps(inputs)
    nc = get_program(Mmat)
    last_err = None
    for _ in range(3):  # rare transient NRT_EXEC_UNIT_UNRECOVERABLE flakes
        try:
            res = run_bass_kernel_spmd(nc, in_maps, list(range(M)))
            return assemble(res.results, pos_list)
        except Exception as e:  # noqa: BLE001
            last_err = e
    raise last_err


# revision 29
# speedup vs baseline: 1.3380x; 1.3380x over previous
"""DenseGINConv on 8 TRN2 NeuronCores (v4: superblock gathers, big SWDGE calls).

  agg = segment_sum(x[edge_src], edge_dst, N)        # gather + scatter-add
  h   = (1+eps)*x + agg
  out = relu(relu(relu(h @ W1 + b1) @ W2 + b2) + bias)

Strategy (fully SPMD, zero collectives):
  - Shard edges by dst range: core i owns dst nodes [i*12500, (i+1)*12500).
  - Replicate x as an fp16 gather table in every core's HBM; gather src rows
    with the dma_gather GPSIMD ucode (int16 indices -> 4 table chunks).
  - Dst nodes are balanced into NBLK=100 blocks of 128 slots, grouped into
    5 superblocks of 20 blocks. Edges are ordered (super, chunk, block) and
    padded to 128-edge groups per (block, chunk).
  - v3 issued one small dma_gather per (block, chunk) [416 calls/core]; the
    ~1us fixed SWDGE cost per call made GPSIMD the bottleneck (~486us busy
    in TimelineSim). v4 enlarges the descriptor ring
    (dynamic_dma_scratch_size=32768 -> 2048-desc rings, 4MB SBUF) and issues
    14-column (1792-edge) calls spanning blocks within a (super, chunk)
    piece: ~120 calls/core, ~190us GPSIMD, leaving the DMA engines (~330us)
    as the bottleneck.
  - PSUM is bank-granular: each of 5 banks holds a [128, 512] f32 tile = 4
    blocks' agg^T accumulated via matmul sub-range outputs (subtile deps).
    One-hot columns are built per piece in one tensor_tensor (is_equal vs
    iota); pieces alternate between DVE and GPSIMD to balance engine load.
  - After a super's last piece: hT = agg + (1+eps)x added in place into the
    fp16 xsT buffer (DVE), then the fp16 2-layer MLP + final relu per block,
    overlapped with the next super's gathers. outT is written fp16 and
    upcast on the host.
"""

import math

import numpy as np

import concourse.bacc as bacc
import concourse.mybir as mybir
import concourse.tile as tile
from concourse.bass_utils import run_bass_kernel_spmd
from concourse.library_config import mlp as mlp_lib

N = 100000
C = 128
M = 8            # cores
NPC = N // M     # nodes per core = 12500
BLK = 128                       # dst slots per block
NBLK = 104                      # dst blocks / core (6.5% cap slack)
SUPER = 13                      # blocks per superblock (4 psum bank tiles)
NSUP = NBLK // SUPER            # 8 superblocks
QUAD = 4                        # blocks per psum bank tile
SLOTS = NBLK * BLK              # padded dst slots / core = 13312
P = 128
NCH = 4                         # x-table chunks (int16 index range)
CH = math.ceil((N + 1) / NCH)   # rows per chunk (25001 <= 32768)
MAXCOLS_CALL = 7                # cols per dma_gather
SCRATCH = 16384                 # dynamic_dma_scratch_size (default)
LIGHT = 8                       # trailing blocks with 3-col (384-edge) caps

f32 = mybir.dt.float32
f16 = mybir.dt.float16
i16 = mybir.dt.int16

_cache = {}


def build(Mmat):
    """Build the per-core Bass program. Mmat[b][c] = 128-edge columns for
    (dst-block b, x-chunk c); identical across cores."""
    nc = bacc.Bacc(
        "TRN2", target_bir_lowering=False, debug=False, enable_asserts=True,
        num_swdge_queues=4, dynamic_dma_scratch_size=SCRATCH,
    )
    totcol = int(Mmat.sum())
    sum16 = totcol * 8  # idx columns (int16, 16-wrapped): 128/16 per column

    xt = nc.dram_tensor("xt", [NCH * CH, C], f16, kind="ExternalInput")
    srcs = nc.dram_tensor("srcs", [P, sum16], i16, kind="ExternalInput")
    dstl = nc.dram_tensor("dstl", [P, totcol], f16, kind="ExternalInput")
    xsT = nc.dram_tensor("xsT", [P, SLOTS], f16, kind="ExternalInput")
    w1 = nc.dram_tensor("W1", [C, C], f16, kind="ExternalInput")
    w2 = nc.dram_tensor("W2", [C, C], f16, kind="ExternalInput")
    b1 = nc.dram_tensor("b1c", [C, 1], f32, kind="ExternalInput")
    b2 = nc.dram_tensor("b2c", [C, 1], f32, kind="ExternalInput")
    bias = nc.dram_tensor("biasc", [C, 1], f32, kind="ExternalInput")
    iota = nc.dram_tensor("iota", [P, BLK], f16, kind="ExternalInput")
    outT = nc.dram_tensor("outT", [P, SLOTS], f16, kind="ExternalOutput")

    piece_cols = {
        (s, c): int(sum(Mmat[b][c] for b in range(s * SUPER, (s + 1) * SUPER)))
        for s in range(NSUP)
        for c in range(NCH)
    }
    maxpc = max(piece_cols.values())

    with tile.TileContext(nc) as tc:
        with (
            tc.tile_pool(name="const", bufs=1) as cp,
            tc.tile_pool(name="gath", bufs=3) as gp,
            tc.tile_pool(name="oh", bufs=3) as op,
            tc.tile_pool(name="mlp", bufs=3) as mp,
            tc.tile_pool(name="psA", bufs=1, space="PSUM") as psA,
            tc.tile_pool(name="psB", bufs=1, space="PSUM") as psB,
            tc.tile_pool(name="psC", bufs=1, space="PSUM") as psC,
        ):
            nc.gpsimd.load_library(mlp_lib)
            srcs_sb = cp.tile([P, sum16], i16)
            nc.sync.dma_start(srcs_sb[:], srcs[:])
            dstl_sb = cp.tile([P, totcol], f16)
            nc.sync.dma_start(dstl_sb[:], dstl[:])
            # xsT is loaded per-super inside the loop (it is only needed for
            # the hT add after a super's gathers), keeping the startup DMA
            # off the gather critical path.
            xsT_sb = cp.tile([P, SLOTS], f16)
            w1_sb = cp.tile([C, C], f16)
            nc.sync.dma_start(w1_sb[:], w1[:])
            w2_sb = cp.tile([C, C], f16)
            nc.sync.dma_start(w2_sb[:], w2[:])
            b1_sb = cp.tile([C, 1], f32)
            nc.sync.dma_start(b1_sb[:], b1[:])
            b2_sb = cp.tile([C, 1], f32)
            nc.sync.dma_start(b2_sb[:], b2[:])
            bias_sb = cp.tile([C, 1], f32)
            nc.sync.dma_start(bias_sb[:], bias[:])
            iota_sb = cp.tile([P, BLK], f16)
            nc.sync.dma_start(iota_sb[:], iota[:])

            col = 0      # global column counter (piece-ordered layout)
            seg16 = 0    # idx column counter
            qn = 0
            for s in range(NSUP):
                blocks = range(s * SUPER, (s + 1) * SUPER)
                # PSUM zero regions are bank-wide: start/stop once per quad
                # bank per super (first/last matmul into it); lanes auto-zero
                # on first touch via the pending-zero mechanism.
                nquad = (SUPER + QUAD - 1) // QUAD
                qtot = {
                    q: int(
                        sum(
                            Mmat[b][c]
                            for c in range(NCH)
                            for b in blocks
                            if (b - s * SUPER) // QUAD == q
                        )
                    )
                    for q in range(nquad)
                }
                qseen = {q: 0 for q in range(nquad)}
                scols = slice(s * SUPER * BLK, (s + 1) * SUPER * BLK)
                nc.sync.dma_start(xsT_sb[:, scols], xsT[:, scols])
                # 4 bank tiles of <=4 blocks each; block b -> quad q, lane l
                quads = {}
                for q in range(nquad):
                    quads[q] = psA.tile(
                        [P, QUAD * BLK], f32, tag=f"aggq{q}", name=f"aggq{q}"
                    )
                for c in range(NCH):
                    pc = piece_cols[(s, c)]
                    if pc == 0:
                        continue
                    gb = gp.tile([P, maxpc * C], f16, tag="g")
                    ncall = -(-pc // MAXCOLS_CALL)
                    splits = [
                        pc * (i + 1) // ncall - pc * i // ncall
                        for i in range(ncall)
                    ]
                    coff = 0
                    for mk in splits:
                        ni = mk * 128
                        nc.gpsimd.dma_gather(
                            gb[:, coff * C:(coff + mk) * C].rearrange(
                                "p (k e) -> p k e", e=C
                            ),
                            xt[c * CH:(c + 1) * CH, :],
                            srcs_sb[:, seg16:seg16 + ni // 16],
                            ni, ni, C, queue_num=qn % 4,
                        )
                        qn += 1
                        seg16 += ni // 16
                        coff += mk
                    oh = op.tile([P, maxpc * BLK], f16, tag="oh")
                    # (is_equal TensorTensor is DVE-only: the Pool engine
                    # fails the neuronxcc ISA opcode check for it.)
                    nc.vector.tensor_tensor(
                        out=oh[:, :pc * BLK].rearrange("p (m e) -> p m e", e=BLK),
                        in0=dstl_sb[:, col:col + pc]
                        .rearrange("p (m o) -> p m o", o=1)
                        .to_broadcast([P, pc, BLK]),
                        in1=iota_sb[:]
                        .rearrange("p (o e) -> p o e", o=1)
                        .to_broadcast([P, pc, BLK]),
                        op=mybir.AluOpType.is_equal,
                    )
                    goff = 0
                    for b in blocks:
                        mb = int(Mmat[b][c])
                        bl = b - s * SUPER
                        q, lane = bl // QUAD, bl % QUAD
                        for j in range(mb):
                            nc.tensor.matmul(
                                out=quads[q][:, lane * BLK:(lane + 1) * BLK],
                                lhsT=gb[:, (goff + j) * C:(goff + j + 1) * C],
                                rhs=oh[:, (goff + j) * BLK:(goff + j + 1) * BLK],
                                start=(qseen[q] == 0),
                                stop=(qseen[q] == qtot[q] - 1),
                            )
                            qseen[q] += 1
                        goff += mb
                    col += pc
                # hT = agg + (1+eps) x, in place over xsT; then MLP per block
                ps1 = ps2 = None
                for b in blocks:
                    bl = b - s * SUPER
                    q, lane = bl // QUAD, bl % QUAD
                    bcols = slice(b * BLK, (b + 1) * BLK)
                    nc.vector.tensor_add(
                        out=xsT_sb[:, bcols],
                        in0=quads[q][:, lane * BLK:(lane + 1) * BLK],
                        in1=xsT_sb[:, bcols],
                    )
                    if lane == 0:
                        ps1 = psB.tile([P, QUAD * BLK], f32, tag="ps1")
                        ps2 = psC.tile([P, QUAD * BLK], f32, tag="ps2")
                    p1 = ps1[:, lane * BLK:(lane + 1) * BLK]
                    nc.tensor.matmul(
                        out=p1, lhsT=w1_sb[:], rhs=xsT_sb[:, bcols],
                        start=True, stop=True,
                    )
                    h1 = mp.tile([P, BLK], f16, tag="h1")
                    nc.scalar.activation(
                        h1[:], p1, mybir.ActivationFunctionType.Relu,
                        bias=b1_sb[:],
                    )
                    p2 = ps2[:, lane * BLK:(lane + 1) * BLK]
                    nc.tensor.matmul(
                        out=p2, lhsT=w2_sb[:], rhs=h1[:], start=True, stop=True
                    )
                    h2 = mp.tile([P, BLK], f16, tag="h2")
                    nc.scalar.activation(
                        h2[:], p2, mybir.ActivationFunctionType.Relu,
                        bias=b2_sb[:],
                    )
                    ob = mp.tile([P, BLK], f16, tag="ob")
                    nc.scalar.activation(
                        ob[:], h2[:], mybir.ActivationFunctionType.Relu,
                        bias=bias_sb[:],
                    )
                    nc.sync.dma_start(out=outT[:, bcols], in_=ob[:])

    nc.compile()
    return nc


def prep(x, edge_src, edge_dst, eps):
    """Host-side sharding -> per-core (srcs16, dstl, xsT) + shared table/M."""
    x = np.asarray(x, dtype=np.float32)
    edge_src = np.asarray(edge_src).astype(np.int64)
    edge_dst = np.asarray(edge_dst).astype(np.int64)
    epsv = float(np.asarray(eps).reshape(-1)[0])

    core = edge_dst // NPC
    dst_local = edge_dst - core * NPC
    chunk = edge_src // CH
    lidx = (edge_src - chunk * CH).astype(np.int16)

    percore = []
    pos_list = []
    counts = np.zeros((M, NBLK, NCH), dtype=np.int64)
    for i in range(M):
        sel = core == i
        dl, c_i = dst_local[sel], chunk[sel]
        deg = np.bincount(dl * NCH + c_i, minlength=NPC * NCH).reshape(NPC, NCH)
        # Mostly 4-column (512-edge) caps per (block, chunk), with LIGHT
        # trailing blocks capped at 3 columns: Mmat is the max over cores, so
        # shared sub-512 quotas shave ~4% of the padded gather volume.
        caps = np.full((NBLK, NCH), 4 * 128, dtype=np.int64)
        caps[NBLK - LIGHT:] = 3 * 128
        dblk, dslot = _balance(deg, NBLK, BLK, caps)
        pos_list.append(dblk * BLK + dslot)
        b_i = dblk[dl]
        slot_i = dslot[dl]
        sup_i = b_i // SUPER
        order = np.lexsort((b_i, c_i, sup_i))   # (super, chunk, block)
        percore.append((lidx[sel][order], slot_i[order],
                        b_i[order], c_i[order]))
        cnt = np.bincount(b_i * NCH + c_i, minlength=NBLK * NCH)
        counts[i] = cnt.reshape(NBLK, NCH)

    Mmat = np.ceil(counts.max(axis=0) / 128).astype(np.int64)  # [NBLK, NCH]
    totcol = int(Mmat.sum())

    # column-start offset of each (b, c) in the (super, chunk, block) layout
    ksort = np.array(
        [b * NCH + c
         for s in range(NSUP)
         for c in range(NCH)
         for b in range(s * SUPER, (s + 1) * SUPER)],
        dtype=np.int64,
    )
    colstart = np.zeros(NBLK * NCH, dtype=np.int64)
    acc = 0
    for kk in ksort:
        colstart[kk] = acc
        acc += Mmat.reshape(-1)[kk]
    assert acc == totcol

    srcs_list, dstl_list, xsT_list = [], [], []
    for i in range(M):
        li, sl, b_i, c_i = percore[i]
        key = b_i * NCH + c_i
        kcnt = counts[i].reshape(-1)
        # edges are sorted (super, chunk, block): cumulate in that order
        pos_in_sorted = np.zeros(NBLK * NCH, dtype=np.int64)
        run = 0
        for kk in ksort:
            pos_in_sorted[kk] = run
            run += kcnt[kk]
        pos = np.arange(len(li)) - pos_in_sorted[key]
        gpos = colstart[key] * 128 + pos  # position in the padded edge stream

        v = np.zeros(totcol * 128, dtype=np.int16)   # pad: row 0 of chunk
        d = np.full(totcol * 128, -1.0, dtype=np.float16)
        v[gpos] = li
        d[gpos] = sl

        # idx stream wraps per 16 within each call; calls are whole columns
        # and 128 % 16 == 0, so the wrap is stream-global as before.
        w = v.reshape(-1, 16).T.copy()               # [16, totcol*8]
        srcs_list.append(np.tile(w, (8, 1)))
        dstl_list.append(
            np.ascontiguousarray(d.reshape(totcol, 128).T)  # [128, totcol]
        )
        xs = np.zeros((P, SLOTS), dtype=np.float16)
        xs[:, pos_list[i]] = (
            ((1.0 + epsv) * x[i * NPC:(i + 1) * NPC]).astype(np.float16).T
        )
        xsT_list.append(xs)

    xt = np.zeros((NCH * CH, C), dtype=np.float16)
    xt[:N] = x
    return Mmat, srcs_list, dstl_list, xsT_list, xt, pos_list


def _balance(deg, nbins, cap_slots, cap_edges):
    """Best-fit-decreasing: assign dsts (rows of deg [ND, NCH]) to nbins
    blocks, <= cap_slots dsts and (soft) <= cap_edges[b, c] edges each."""
    nd = deg.shape[0]
    tot = deg.sum(axis=1)
    order = np.argsort(-tot, kind="stable")
    sums = np.zeros((nbins, deg.shape[1]), dtype=np.int64)
    load = np.zeros(nbins, dtype=np.int64)
    cnt = np.zeros(nbins, dtype=np.int64)
    blk = np.empty(nd, dtype=np.int64)
    slot = np.empty(nd, dtype=np.int64)
    big = 1 << 50
    for d in order:
        v = deg[d]
        ok = (cnt < cap_slots) & ((sums + v) <= cap_edges).all(axis=1)
        if ok.any():
            b = int(np.argmin(np.where(ok, load, big)))  # LPT: least-loaded fit
        else:
            over = np.maximum(sums + v - cap_edges, 0).sum(axis=1)
            over[cnt >= cap_slots] = big
            b = int(np.argmin(over))
        blk[d] = b
        slot[d] = cnt[b]
        cnt[b] += 1
        load[b] += tot[d]
        sums[b] += v
    return blk, slot


def make_in_maps(inputs):
    Mmat, srcs_list, dstl_list, xsT_list, xt, pos_list = prep(
        inputs["x"], inputs["edge_src"], inputs["edge_dst"], inputs["eps"]
    )
    w1 = np.ascontiguousarray(np.asarray(inputs["W1"], dtype=np.float16))
    w2 = np.ascontiguousarray(np.asarray(inputs["W2"], dtype=np.float16))
    b1c = np.asarray(inputs["b1"], dtype=np.float32).reshape(C, 1)
    b2c = np.asarray(inputs["b2"], dtype=np.float32).reshape(C, 1)
    biasc = np.asarray(inputs["bias"], dtype=np.float32).reshape(C, 1)
    iota = np.tile(np.arange(BLK, dtype=np.float16), (P, 1))
    in_maps = [
        dict(
            xt=xt, srcs=srcs_list[i], dstl=dstl_list[i], xsT=xsT_list[i],
            W1=w1, W2=w2, b1c=b1c, b2c=b2c, biasc=biasc, iota=iota,
        )
        for i in range(M)
    ]
    return Mmat, in_maps, pos_list


def get_program(Mmat):
    key = Mmat.tobytes()
    if key not in _cache:
        _cache[key] = build(Mmat)
    return _cache[key]


def assemble(results, pos_list):
    out = np.empty((N, C), dtype=np.float32)
    for i in range(M):
        out[i * NPC:(i + 1) * NPC] = (
            results[i]["outT"].astype(np.float32).T[pos_list[i]]
        )
    return out


def kernel(**inputs) -> np.ndarray:
    Mmat, in_maps, pos_list = make_in_maps(inputs)
    nc = get_program(Mmat)
    last_err = None
    for _ in range(3):  # rare transient NRT_EXEC_UNIT_UNRECOVERABLE flakes
        try:
            res = run_bass_kernel_spmd(nc, in_maps, list(range(M)))
            return assemble(res.results, pos_list)
        except Exception as e:  # noqa: BLE001
            last_err = e
    raise last_err
